# revision 1
# baseline (speedup 1.0000x reference)
"""Co-Guiding GAT forward (2 layers, 4 masked-MHA branches) on 8 Trainium2 cores.

Sharding: core c = 2*b + p handles batch b; p=0 computes the h_a stream
(branches a2a + b2a), p=1 the h_b stream (b2b + a2b). Each core runs both
layers; the h exchange between layers is a pairwise AllGather of the
transposed fp16 activations, overlapped with the partner-independent half of
layer 2.

Attention is computed in transposed score layout S^T[k, q] per head so softmax
needs no transposes: exp on ACT straight out of PSUM (with a constant shift to
keep fp16 in range; softmax is shift-invariant), multiplicative {0,1}
adjacency mask on DVE/GPSIMD (fp16), row sums via an appended ones-column in
the V matmul, and per-head normalization folded into the PSUM eviction.
"""

import sys

for _p in ("/opt/trn_rl_repo",):
    if _p not in sys.path:
        sys.path.insert(0, _p)

import math

import numpy as np

import concourse.bass as bass
import concourse.mybir as mybir
import concourse.tile as tile
from concourse import bacc
from concourse.bass_utils import run_bass_kernel_spmd
from concourse.masks import make_identity

F32 = mybir.dt.float32
F16 = mybir.dt.float16
AF = mybir.ActivationFunctionType
OP = mybir.AluOpType

N_CORES = 8
B, N, D = 4, 1024, 256
H, DK = 8, 32
L = 2
P = 128
NT = N // P  # 8 row tiles
DT = D // P  # 2 feature tiles
EPS = 1e-5
SCALE = 1.0 / math.sqrt(DK)
SHIFT = -12.0  # exp(s/sqrt(dk) + SHIFT): keeps p in fp16 range

_CACHED_NC = None
_LAST_IN_MAPS = None


def build_nc(finalize=True, dbg=False):
    nc = bacc.Bacc("TRN2", target_bir_lowering=False, debug=False,
                   num_devices=N_CORES)

    # ---- per-core DRAM I/O ----
    x_d = nc.dram_tensor("x", [N, D], F32, kind="ExternalInput")
    xT_d = nc.dram_tensor("xT", [D, N], F16, kind="ExternalInput")
    yT_d = nc.dram_tensor("yT", [D, N], F16, kind="ExternalInput")
    mks_d = nc.dram_tensor("maskTs", [N, N], F16, kind="ExternalInput")
    mkc_d = nc.dram_tensor("maskTc", [N, N], F16, kind="ExternalInput")
    # weights packed [l, role, mat(q,k,v,o), kc, 128, dout]
    wts_d = nc.dram_tensor("wts", [L, 2, 4, DT, P, D], F16, kind="ExternalInput")
    brow_d = nc.dram_tensor("brow", [1, L, 2, 4, D], F16, kind="ExternalInput")
    lng_d = nc.dram_tensor("lng", [L, 2, D], F32, kind="ExternalInput")
    lnb_d = nc.dram_tensor("lnb", [L, 2, D], F32, kind="ExternalInput")
    sel_d = nc.dram_tensor("sel", [P, 2], F32, kind="ExternalInput")
    out_d = nc.dram_tensor("out", [N, D], F32, kind="ExternalOutput")
    if dbg:
        dbg_qT = nc.dram_tensor("dbg_qT", [D, N], F16, kind="ExternalOutput")
        dbg_kT = nc.dram_tensor("dbg_kT", [D, N], F16, kind="ExternalOutput")
        dbg_v = nc.dram_tensor("dbg_v", [N, H * (DK + 1)], F16, kind="ExternalOutput")
        dbg_a0 = nc.dram_tensor("dbg_a0", [D, N], F16, kind="ExternalOutput")
        dbg_a1 = nc.dram_tensor("dbg_a1", [D, N], F16, kind="ExternalOutput")
        dbg_x1 = nc.dram_tensor("dbg_x1", [N, D], F32, kind="ExternalOutput")

    with tile.TileContext(nc) as tc:
        with (
            tc.tile_pool(name="const", bufs=1) as cpool,
            tc.tile_pool(name="wts", bufs=2) as wpool,
            tc.tile_pool(name="trans", bufs=1) as tpool,
            tc.tile_pool(name="nat", bufs=2) as npool,
            tc.tile_pool(name="qk", bufs=3) as qkpool,
            tc.tile_pool(name="vsb", bufs=2) as vpool,
            tc.tile_pool(name="pt", bufs=2) as ptpool,
            tc.tile_pool(name="att", bufs=2) as apool,
            tc.tile_pool(name="rb", bufs=2) as rbpool,
            tc.tile_pool(name="sm", bufs=2) as smpool,
            tc.tile_pool(name="ln", bufs=2) as lnpool,
            tc.tile_pool(name="pbig", bufs=2, space="PSUM") as pbig,
            tc.tile_pool(name="pout", bufs=2, space="PSUM") as pout,
            tc.tile_pool(name="dram", bufs=2, space="DRAM") as dpool,
        ):
            # ---------- constants / resident loads ----------
            ident = cpool.tile([P, P], F32, tag="ident")
            make_identity(nc, ident)
            ones = cpool.tile([1, 512], F16, tag="ones")
            nc.vector.memset(ones, 1.0)
            shift = cpool.tile([P, 1], F32, tag="shift")
            nc.vector.memset(shift, SHIFT)
            epst = cpool.tile([P, 1], F32, tag="epst")
            nc.vector.memset(epst, EPS)

            mask_sb = {}
            for role, md in ((0, mks_d), (1, mkc_d)):
                m = cpool.tile([P, NT, N], F16, tag=f"mask{role}")
                nc.sync.dma_start(out=m, in_=md.ap().rearrange(
                    "(kc p) q -> p kc q", p=P))
                mask_sb[role] = m

            brow = cpool.tile([1, L, 2, 4, D], F16, tag="brow")
            nc.sync.dma_start(out=brow, in_=brow_d.ap())
            sel = cpool.tile([P, 2], F32, tag="sel")
            nc.sync.dma_start(out=sel, in_=sel_d.ap())

            gB = cpool.tile([P, L, 2, D], F32, tag="gB")
            bB = cpool.tile([P, L, 2, D], F32, tag="bB")
            for t, src in ((gB, lng_d), (bB, lnb_d)):
                bc = bass.AP(tensor=src.ap().tensor, offset=0,
                             ap=[[0, P]] + list(src.ap().ap))
                nc.gpsimd.dma_start(out=t, in_=bc)

            orig_x = cpool.tile([P, NT, D], F32, tag="orig_x")
            nc.sync.dma_start(out=orig_x, in_=x_d.ap().rearrange(
                "(nt p) d -> p nt d", p=P))

            xT1 = tpool.tile([P, DT, N], F16, tag="xT")
            nc.sync.dma_start(out=xT1, in_=xT_d.ap().rearrange(
                "(dt p) q -> p dt q", p=P))
            yT1 = tpool.tile([P, DT, N], F16, tag="yT")
            nc.sync.dma_start(out=yT1, in_=yT_d.ap().rearrange(
                "(dt p) q -> p dt q", p=P))

            # ---------------- helpers ----------------
            def projections(l, role, xT, kvT, w):
                """qT/kT [dout, q] fp16, v_sb [n, h, dk+1] fp16 (ones col)."""
                qT = qkpool.tile([P, DT, N], F16, tag="qk")
                kT = qkpool.tile([P, DT, N], F16, tag="qk")
                for dst, m_i, src in ((qT, 0, xT), (kT, 1, kvT)):
                    for mc in range(DT):
                        ps = pbig.tile([P, N], F32, tag="pb")
                        for qc in range(2):
                            o = ps[:, qc * 512:(qc + 1) * 512]
                            for kc in range(DT):
                                nc.tensor.matmul(
                                    o, w[:, m_i, kc, mc * P:(mc + 1) * P],
                                    src[:, kc, qc * 512:(qc + 1) * 512],
                                    start=(kc == 0), stop=False)
                            nc.tensor.matmul(
                                o, brow[0:1, l, role, m_i, mc * P:(mc + 1) * P],
                                ones[0:1, 0:512], start=False, stop=True)
                        nc.vector.tensor_copy(out=dst[:, mc, :], in_=ps)
                v_sb = vpool.tile([P, NT, H, DK + 1], F16, tag="v")
                nc.gpsimd.memset(v_sb[:, :, :, DK:DK + 1], 1.0)
                for g4 in range(2):
                    ps = pbig.tile([P, N], F32, tag="pb")
                    for sub in range(4):
                        nt = g4 * 4 + sub
                        o = ps[:, sub * D:(sub + 1) * D]
                        for kc in range(DT):
                            nc.tensor.matmul(
                                o, kvT[:, kc, nt * P:(nt + 1) * P],
                                w[:, 2, kc, :], start=(kc == 0), stop=False)
                        nc.tensor.matmul(
                            o, ones[0:1, 0:P], brow[0:1, l, role, 2, :],
                            start=False, stop=True)
                    nc.vector.tensor_copy(
                        out=v_sb[:, g4 * 4:(g4 + 1) * 4, :, 0:DK],
                        in_=ps.rearrange("p (s h d) -> p s h d", s=4, h=H))
                return qT, kT, v_sb

            def attention(role, qT, kT, v_sb):
                """Masked softmax attention; returns normalized attnT fp16."""
                maskT = mask_sb[role]
                araw = apool.tile([P, DT, N], F16, tag="attnT")
                for hp in range(4):
                    ops = pout.tile([P, N], F32, tag="po")
                    for hh in range(2):
                        h = hp * 2 + hh
                        th, oh = h // 4, (h % 4) * DK

                        def st_tile(kc):
                            sps = pbig.tile([P, N], F32, tag="pb")
                            for qc in range(2):
                                nc.tensor.matmul(
                                    sps[:, qc * 512:(qc + 1) * 512],
                                    kT[oh:oh + DK, th, kc * P:(kc + 1) * P],
                                    qT[oh:oh + DK, th, qc * 512:(qc + 1) * 512],
                                    start=True, stop=True, tile_position=(oh, 0))
                            pt = ptpool.tile([P, N], F16, tag="pt")
                            nc.scalar.activation(out=pt, in_=sps, func=AF.Exp,
                                                 scale=SCALE, bias=shift)
                            eng = nc.vector if (h * NT + kc) % 2 == 0 else nc.gpsimd
                            eng.tensor_mul(pt, pt, maskT[:, kc, :])
                            return pt

                        def av(kc, pt):
                            for qc in range(2):
                                nc.tensor.matmul(
                                    ops[hh * 64:hh * 64 + 33,
                                        qc * 512:(qc + 1) * 512],
                                    v_sb[:, kc, h, :],
                                    pt[:, qc * 512:(qc + 1) * 512],
                                    start=(kc == 0), stop=(kc == NT - 1))

                        # depth-1 software pipeline: S^T(kc+1) issues on PE
                        # before AV(kc), so PE never waits on the exp/mask
                        prev = st_tile(0)
                        for kc in range(1, NT):
                            cur = st_tile(kc)
                            av(kc - 1, prev)
                            prev = cur
                        av(NT - 1, prev)

                    rsd = dpool.tile([2, N], F32, tag="rsd")
                    for hh in range(2):
                        h = hp * 2 + hh
                        th, oh = h // 4, (h % 4) * DK
                        srow = smpool.tile([1, N], F32, tag="srow")
                        if hh == 0:
                            nc.scalar.copy(out=srow, in_=ops[32:33, :])
                        else:
                            nc.vector.tensor_copy(out=srow, in_=ops[96:97, :])
                        rs = smpool.tile([1, N], F32, tag="rs")
                        nc.vector.reciprocal_approx_fast(out=rs, in_=srow)
                        nc.sync.dma_start(out=rsd[hh:hh + 1, :], in_=rs)
                        rb = rbpool.tile([DK, 1, N], F32, tag="rb")
                        nc.sync.dma_start(
                            out=rb, in_=rsd[hh:hh + 1, :].partition_broadcast(DK))
                        nc.vector.scalar_tensor_tensor(
                            out=araw[oh:oh + DK, th, :],
                            in0=ops[hh * 64:hh * 64 + 32, :],
                            scalar=1.0, in1=rb[:, 0, :],
                            op0=OP.mult, op1=OP.mult)
                return araw

            def outproj_ln(l, role, araw, w, x_nat, xacc, xnew):
                """out-proj + bias + residual + LayerNorm; role 0 fills xacc,
                role 1 combines into xnew (relu, plus orig_x residual at l=1)."""
                for nt in range(NT):
                    ops = pout.tile([P, N], F32, tag="po")
                    o = ops[:, 0:D]
                    for kc2 in range(DT):
                        nc.tensor.matmul(
                            o, araw[:, kc2, nt * P:(nt + 1) * P],
                            w[:, 3, kc2, :], start=(kc2 == 0), stop=False)
                    nc.tensor.matmul(
                        o, ones[0:1, 0:P], brow[0:1, l, role, 3, :],
                        start=False, stop=True)
                    t = lnpool.tile([P, D], F32, tag="t")
                    nc.vector.tensor_add(t, o, x_nat[:, nt, :])
                    st = lnpool.tile([P, 6], F32, tag="st")
                    nc.vector.bn_stats(out=st, in_=t)
                    mv = lnpool.tile([P, 2], F32, tag="mv")
                    nc.vector.bn_aggr(out=mv, in_=st)
                    rstd = lnpool.tile([P, 1], F32, tag="rstd")
                    nc.scalar.activation(out=rstd, in_=mv[:, 1:2],
                                         func=AF.Sqrt, bias=epst)
                    nc.vector.reciprocal(out=rstd, in_=rstd)
                    tn = lnpool.tile([P, D], F32, tag="tn")
                    nc.vector.tensor_scalar(
                        out=tn, in0=t, scalar1=mv[:, 0:1], scalar2=rstd,
                        op0=OP.subtract, op1=OP.mult)
                    u = lnpool.tile([P, D], F32, tag="u")
                    nc.vector.tensor_mul(u, tn, gB[:, l, role, :])
                    if role == 0:
                        nc.vector.tensor_add(xacc[:, nt, :], u, bB[:, l, role, :])
                    else:
                        v1 = lnpool.tile([P, D], F32, tag="v1")
                        nc.vector.tensor_add(v1, u, bB[:, l, role, :])
                        pre = lnpool.tile([P, D], F32, tag="pre")
                        nc.vector.tensor_add(pre, v1, xacc[:, nt, :])
                        if l == 0:
                            nc.vector.tensor_scalar_max(xnew[:, nt, :], pre, 0.0)
                        else:
                            nc.vector.scalar_tensor_tensor(
                                out=xnew[:, nt, :], in0=pre, scalar=0.0,
                                in1=orig_x[:, nt, :], op0=OP.max, op1=OP.add)
                            nc.sync.dma_start(
                                out=out_d.ap().rearrange(
                                    "(nt2 p) d -> p nt2 d", p=P)[:, nt, :],
                                in_=xnew[:, nt, :])

            # ---------------- layers ----------------
            x_nat, xT, yT = orig_x, xT1, yT1
            xgT_out = None
            for l in range(L):
                w0 = wpool.tile([P, 4, DT, D], F16, tag="w")
                nc.sync.dma_start(out=w0, in_=wts_d.ap()[l, 0].rearrange(
                    "m kc p d -> p m kc d"))
                qT0, kT0, v0 = projections(l, 0, xT, xT, w0)
                if dbg and l == 0:
                    nc.sync.dma_start(out=dbg_qT.ap().rearrange(
                        "(dt p) q -> p dt q", p=P), in_=qT0)
                    nc.sync.dma_start(out=dbg_kT.ap().rearrange(
                        "(dt p) q -> p dt q", p=P), in_=kT0)
                    nc.sync.dma_start(out=dbg_v.ap().rearrange(
                        "(nt p) e -> p nt e", p=P),
                        in_=v0.rearrange("p nt h e -> p nt (h e)"))
                a0 = attention(0, qT0, kT0, v0)
                if dbg and l == 0:
                    nc.sync.dma_start(out=dbg_a0.ap().rearrange(
                        "(dt p) q -> p dt q", p=P), in_=a0)

                if l == 1:
                    # partner's transposed activations arrive via AllGather;
                    # select our partner's half (sel is 0/1 per core parity)
                    g0T = qkpool.tile([P, DT, N], F16, tag="qk")
                    g1T = qkpool.tile([P, DT, N], F16, tag="qk")
                    nc.sync.dma_start(out=g0T, in_=xgT_out[0:D, :].rearrange(
                        "(dt p) q -> p dt q", p=P))
                    nc.sync.dma_start(out=g1T, in_=xgT_out[D:2 * D, :].rearrange(
                        "(dt p) q -> p dt q", p=P))
                    yT2 = tpool.tile([P, DT, N], F16, tag="yT")
                    nc.vector.tensor_scalar_mul(yT2, g0T, sel[:, 0:1])
                    nc.vector.scalar_tensor_tensor(
                        out=yT2, in0=g1T, scalar=sel[:, 1:2], in1=yT2,
                        op0=OP.mult, op1=OP.add)
                    yT = yT2

                w1 = wpool.tile([P, 4, DT, D], F16, tag="w")
                nc.sync.dma_start(out=w1, in_=wts_d.ap()[l, 1].rearrange(
                    "m kc p d -> p m kc d"))
                qT1, kT1, v1 = projections(l, 1, xT, yT, w1)

                xacc = npool.tile([P, NT, D], F32, tag="xacc")
                xnew = npool.tile([P, NT, D], F32, tag="xnew")
                outproj_ln(l, 0, a0, w0, x_nat, xacc, xnew)

                a1 = attention(1, qT1, kT1, v1)
                if dbg and l == 0:
                    nc.sync.dma_start(out=dbg_a1.ap().rearrange(
                        "(dt p) q -> p dt q", p=P), in_=a1)
                outproj_ln(l, 1, a1, w1, x_nat, xacc, xnew)

                if l == 0:
                    if dbg:
                        nc.sync.dma_start(out=dbg_x1.ap().rearrange(
                            "(nt p) d -> p nt d", p=P), in_=xnew)
                    # transpose x_new (fp16) and exchange with the pair core
                    xT2 = tpool.tile([P, DT, N], F16, tag="xT")
                    for dt_i in range(DT):
                        for g2 in range(2):
                            ps = pout.tile([P, N], F32, tag="po")
                            for s4 in range(4):
                                nt = g2 * 4 + s4
                                nc.tensor.transpose(
                                    ps[:, s4 * P:(s4 + 1) * P],
                                    xnew[:, nt, dt_i * P:(dt_i + 1) * P], ident)
                            nc.vector.tensor_copy(
                                out=xT2[:, dt_i, g2 * 512:(g2 + 1) * 512],
                                in_=ps[:, 0:512])
                    xgT_in = dpool.tile([D, N], F16, tag="xgin")
                    xgT_out = dpool.tile([2 * D, N], F16, tag="xgout")
                    nc.sync.dma_start(
                        out=xgT_in.rearrange("(dt p) q -> p dt q", p=P), in_=xT2)
                    nc.gpsimd.collective_compute(
                        "AllGather", OP.bypass,
                        replica_groups=[[2 * i, 2 * i + 1] for i in range(4)],
                        ins=[xgT_in.opt()], outs=[xgT_out.opt()])
                    x_nat, xT = xnew, xT2

    if finalize:
        nc.finalize()
    return nc


def kernel(h_a, h_b, adj_a, adj_b, adj_ab, adj_ba,
           Wq, bq, Wk, bk, Wv, bv, Wo, bo, ln_g, ln_b):
    global _CACHED_NC, _LAST_IN_MAPS
    h_a = np.asarray(h_a, np.float32)
    h_b = np.asarray(h_b, np.float32)
    arrs = dict(Wq=np.asarray(Wq, np.float32), Wk=np.asarray(Wk, np.float32),
                Wv=np.asarray(Wv, np.float32), Wo=np.asarray(Wo, np.float32),
                bq=np.asarray(bq, np.float32), bk=np.asarray(bk, np.float32),
                bv=np.asarray(bv, np.float32), bo=np.asarray(bo, np.float32),
                ln_g=np.asarray(ln_g, np.float32), ln_b=np.asarray(ln_b, np.float32))
    adjs = dict(a=np.asarray(adj_a), b=np.asarray(adj_b),
                ab=np.asarray(adj_ab), ba=np.asarray(adj_ba))

    if _CACHED_NC is None:
        _CACHED_NC = build_nc()
    nc = _CACHED_NC

    in_maps = []
    for c in range(N_CORES):
        b, p = c // 2, c % 2
        if p == 0:
            x, y = h_a[b], h_b[b]
            mself, mcross = adjs["a"][b], adjs["ba"][b]
            roles = (0, 3)  # a2a, b2a
        else:
            x, y = h_b[b], h_a[b]
            mself, mcross = adjs["b"][b], adjs["ab"][b]
            roles = (1, 2)  # b2b, a2b
        wts = np.empty((L, 2, 4, DT, P, D), np.float16)
        brow = np.empty((1, L, 2, 4, D), np.float16)
        lng = np.empty((L, 2, D), np.float32)
        lnb = np.empty((L, 2, D), np.float32)
        for l in range(L):
            for r, j in enumerate(roles):
                for m, (Wn, bn) in enumerate(
                        (("Wq", "bq"), ("Wk", "bk"), ("Wv", "bv"), ("Wo", "bo"))):
                    wts[l, r, m] = arrs[Wn][l, j].reshape(DT, P, D)
                    brow[0, l, r, m] = arrs[bn][l, j]
                lng[l, r] = arrs["ln_g"][l, j]
                lnb[l, r] = arrs["ln_b"][l, j]
        sel = np.zeros((P, 2), np.float32)
        sel[:, 1 - p] = 1.0  # p=0 wants partner (slot1); p=1 wants slot0
        in_maps.append({
            "x": np.ascontiguousarray(x),
            "xT": np.ascontiguousarray(x.T).astype(np.float16),
            "yT": np.ascontiguousarray(y.T).astype(np.float16),
            "maskTs": np.ascontiguousarray(mself.T).astype(np.float16),
            "maskTc": np.ascontiguousarray(mcross.T).astype(np.float16),
            "wts": wts, "brow": brow, "lng": lng, "lnb": lnb, "sel": sel,
        })

    _LAST_IN_MAPS = in_maps
    res = run_bass_kernel_spmd(nc, in_maps, list(range(N_CORES)))
    out_a = np.stack([res.results[2 * b]["out"] for b in range(B)])
    out_b = np.stack([res.results[2 * b + 1]["out"] for b in range(B)])
    return out_a, out_b

